# revision 1
# baseline (speedup 1.0000x reference)
"""Trainium2 Bass kernel for MCRNNVAE eval forward (nn_MCRNNVAE_34754875359779).

Key insight: the reference network has no nonlinearity other than the RNN tanh
(PhiBlock/VariationalBlock hidden layers are linear), so per channel c the whole
per-timestep chain collapses algebraically to a vanilla RNN:

    h_{t+1} = tanh(x_t @ U_c + h_t @ M_c + ub_c)
    mu_t    = x_t @ V_c + h_t @ N_c + vb_c

with U [F,H], M [H,H], N [H,F], V [F,F] folded on the host in float64.

Sharding: 3 channels x 512 batch = 1536 recurrence columns -> 8 cores x (128+64).
Every core runs the same SPMD program with two recurrence instances (width 128
and width 64); the (channel, batch-slice) assignment is carried entirely by the
per-core input data (weights + host-transposed x slices).

On-device layout is fully transposed (features on partitions, (t, batch) on the
free axis) so every matmul uses host-shipped weights as the stationary operand
in natural layout and the tanh output lands directly in next-step layout.
"""

import os
import numpy as np

import ml_dtypes

C, T, B, F = 3, 100, 512, 128
H = 256
WA, WB = 128, 64  # per-core recurrence widths (columns of batch x channel)

# (channel, b0) for each core's width-128 piece and width-64 piece.
PIECES_A = [(0, 0), (0, 128), (0, 256), (0, 384), (1, 0), (1, 128), (2, 0), (2, 128)]
PIECES_B = [(1, 256), (1, 320), (1, 384), (1, 448),
            (2, 256), (2, 320), (2, 384), (2, 448)]

BF16 = ml_dtypes.bfloat16


def _fold_weights(inputs):
    """Collapse the linear chain per channel, in float64. Returns per-channel
    (U [128,256], ub [256], M [256,256], N [256,128], V [128,128], vb [128])."""
    HX, HZ, EH, L = 128, 128, 128, 64
    g = lambda k: np.asarray(inputs[k], np.float64)
    out = []
    for c in range(C):
        Wx, bx = g("phi_x_W")[c], g("phi_x_b")[c]
        We, be = g("enc_W")[c], g("enc_b")[c]
        Wqm, bqm = g("enc_mu_W")[c], g("enc_mu_b")[c]
        Wz, bz = g("phi_z_W"), g("phi_z_b")
        Wd, bd = g("dec_W")[c], g("dec_b")[c]
        Wpm, bpm = g("dec_mu_W")[c], g("dec_mu_b")[c]
        Wih, Whh = g("rnn_Wih"), g("rnn_Whh")
        bih, bhh = g("rnn_bih"), g("rnn_bhh")

        We_x, We_h = We[:HX], We[HX:]
        Wd_z, Wd_h = Wd[:HZ], Wd[HZ:]
        Wih_x, Wih_z = Wih[:HX], Wih[HX:]

        PWz = Wqm @ Wz                     # [EH, HZ]
        P = We_x @ PWz                     # [HX, HZ]
        Q = We_h @ PWz                     # [H, HZ]
        r = be @ PWz + bqm @ Wz + bz       # [HZ]

        G = Wih_x + P @ Wih_z              # [HX, H]
        M = Q @ Wih_z + Whh                # [H, H]
        gv = r @ Wih_z + bih + bhh         # [H]

        U = Wx @ G                         # [F, H]
        ub = bx @ G + gv                   # [H]

        W2 = Wd_z @ Wpm                    # [HZ, F]
        V = Wx @ (P @ W2)                  # [F, F]
        N = (Q @ Wd_z + Wd_h) @ Wpm        # [H, F]
        vb = bx @ (P @ W2) + r @ W2 + bd @ Wpm + bpm  # [F]
        out.append((U, ub, M, N, V, vb))
    return out


_NC_CACHE = {}


def _build_nc():
    if "nc" in _NC_CACHE:
        return _NC_CACHE["nc"]
    import concourse.bacc as bacc
    import concourse.mybir as mybir
    import concourse.tile as tile

    DT = mybir.dt.bfloat16
    F32 = mybir.dt.float32
    Tanh = mybir.ActivationFunctionType.Tanh
    Add = mybir.AluOpType.add

    nc = bacc.Bacc()

    dram = {}
    for sfx, w in (("a", WA), ("b", WB)):
        dram[f"xT_{sfx}"] = nc.declare_dram_parameter(f"xT_{sfx}", [128, T * w], DT,
                                                      isOutput=False)
        dram[f"wb_{sfx}"] = nc.declare_dram_parameter(f"wb_{sfx}", [128, 1152], DT,
                                                      isOutput=False)
        dram[f"ub_{sfx}"] = nc.declare_dram_parameter(f"ub_{sfx}", [128, 2], F32,
                                                      isOutput=False)
        dram[f"out_{sfx}"] = nc.declare_dram_parameter(
            f"out_{sfx}", [128, T * w], DT, isOutput=True)

    with tile.TileContext(nc) as tc:
        with (
            tc.tile_pool(name="wts", bufs=1) as wpool,
            tc.tile_pool(name="big", bufs=1) as xpool,
            tc.tile_pool(name="mu_out", bufs=1) as mupool,
            tc.tile_pool(name="ps_a", bufs=2, space="PSUM") as ps_a,
            tc.tile_pool(name="ps_b", bufs=2, space="PSUM") as ps_b,
            tc.tile_pool(name="ps_mu", bufs=3, space="PSUM") as ps_mu,
            tc.tile_pool(name="ps_scr", bufs=1, space="PSUM") as ps_scr,
        ):
            inst = {}
            scr = ps_scr.tile([1, 1], mybir.dt.float32, tag="scr", name="scr")
            gscr = wpool.tile([128, 1], mybir.dt.bfloat16, tag="gscr", name="gscr")
            for sfx, w, pspool in (("a", WA, ps_a), ("b", WB, ps_b)):
                d = {}
                # One packed DMA for all bf16 weights of this instance:
                # cols [0:256)=U, [256:512)=M0, [512:768)=M1, [768:896)=N0,
                # [896:1024)=N1, [1024:1152)=V.
                blob = wpool.tile([128, 1152], DT, tag=f"wb{sfx}", name=f"wb{sfx}")
                nc.sync.dma_start(blob[:], dram[f"wb_{sfx}"][:])
                d["U"] = blob[:, 0:256]
                d["M0"] = blob[:, 256:512]
                d["M1"] = blob[:, 512:768]
                d["N0"] = blob[:, 768:896]
                d["N1"] = blob[:, 896:1024]
                d["V"] = blob[:, 1024:1152]
                ubt = wpool.tile([128, 2], F32, tag=f"ub{sfx}", name=f"ub{sfx}")
                nc.sync.dma_start(ubt[:], dram[f"ub_{sfx}"][:])
                d["ub0"] = ubt[:, 0:1]
                d["ub1"] = ubt[:, 1:2]

                # Every compute instruction may carry at most ONE sync wait on
                # this target, so prime each engine on the init DMAs once:
                # a dummy tanh observes the bias DMA on ACT, a dummy matmul
                # observes the weight-blob DMA on PE.
                warm0 = wpool.tile([128, 2], F32, tag=f"wm0{sfx}", name=f"wm0{sfx}")
                nc.scalar.activation(warm0[:], ubt[:], Tanh, bias=d["ub0"])
                nc.tensor.matmul(scr[:], blob[:, 0:1], blob[:, 0:1],
                                 start=True, stop=True)

                cols = T * w
                d["xT"] = xpool.tile([128, cols], DT, tag=f"xT{sfx}", name=f"xT{sfx}")
                nchunk = 4
                cw = cols // nchunk
                for i in range(nchunk):
                    nc.sync.dma_start(d["xT"][:, i * cw:(i + 1) * cw],
                                      dram[f"xT_{sfx}"][:, i * cw:(i + 1) * cw])
                # h state history: block t holds h_t (transposed, bf16).
                # Block 0 (h_0 = 0) is never touched: step 0 and mu chunk 0
                # special-case it away, so no memset is needed.
                d["h0"] = xpool.tile([128, (T + 1) * w], DT, tag=f"h0{sfx}", name=f"h0{sfx}")
                d["h1"] = xpool.tile([128, (T + 1) * w], DT, tag=f"h1{sfx}", name=f"h1{sfx}")
                d["w"] = w
                d["pspool"] = pspool
                inst[sfx] = d

            def rec_step(sfx, t):
                d = inst[sfx]
                w = d["w"]
                ps = d["pspool"].tile([128, 2, w], mybir.dt.float32,
                                      tag=f"ps{sfx}", name=f"ps{sfx}")
                s = slice(t * w, (t + 1) * w)
                for j in (0, 1):
                    js = slice(j * 128, (j + 1) * 128)
                    nc.tensor.matmul(ps[:, j, :], d["U"][:, js], d["xT"][:, s],
                                     start=True, stop=(t == 0))
                    if t > 0:
                        nc.tensor.matmul(ps[:, j, :], d["M0"][:, js], d["h0"][:, s],
                                         start=False, stop=False)
                        nc.tensor.matmul(ps[:, j, :], d["M1"][:, js], d["h1"][:, s],
                                         start=False, stop=True)
                so = slice((t + 1) * w, (t + 2) * w)
                nc.scalar.activation(d["h0"][:, so], ps[:, 0, :], Tanh,
                                     bias=d["ub0"][:])
                nc.scalar.activation(d["h1"][:, so], ps[:, 1, :], Tanh,
                                     bias=d["ub1"][:])

            def mu_chunk(sfx, idx, c0, cw):
                d = inst[sfx]
                w = d["w"]
                ps = ps_mu.tile([128, cw], mybir.dt.float32, tag="psmu", name="psmu")
                cs = slice(c0, c0 + cw)
                # Primer: observe the previous chunk's DVE copy on PE so the
                # V-matmul's psum-slot WAR needs only the PE-self wait.
                prev = inst.get("prev_ot")
                if prev is not None:
                    nc.tensor.matmul(scr[:], prev[:, 0:1], prev[:, 0:1],
                                     start=True, stop=True)
                nc.tensor.matmul(ps[:], d["V"][:], d["xT"][:, cs],
                                 start=True, stop=False)
                # h_0 = 0 and block 0 of h is uninitialized: the first chunk's
                # N-matmuls cover only columns [w:) (mu_0 = mux_0 exactly).
                h0c = c0 + w if c0 == 0 else c0
                hs = slice(h0c, c0 + cw)
                po = slice(h0c - c0, cw)
                nc.tensor.matmul(ps[:, po], d["N0"][:], d["h0"][:, hs],
                                 start=False, stop=False)
                nc.tensor.matmul(ps[:, po], d["N1"][:], d["h1"][:, hs],
                                 start=False, stop=True)
                # DVE copy to a never-reused bf16 staging tile (no WAR -> the
                # copy's single wait is the PE psum dep), then GPSIMD-issued
                # (SWDGE, multi-wait-capable) DMA out. vb is added on the host.
                ot = mupool.tile([128, cw], DT, tag=f"mu{sfx}{idx}",
                                 name=f"mu{sfx}{idx}")
                nc.vector.tensor_copy(ot[:], ps[:])
                # GPSIMD interposer: observe the DVE copy in GPSIMD program
                # order so the out-DMA instruction needs only its queue wait.
                nc.gpsimd.tensor_copy(gscr[:], ot[:, 0:1])
                nc.gpsimd.dma_start(dram[f"out_{sfx}"][:, cs], ot[:])
                inst["prev_ot"] = ot

            # Recurrence steps with mu chunks interleaved as their h inputs
            # become available (A: 512 cols = 4 steps; B: 512 cols = 8 steps).
            for t in range(T):
                if t % 25 == 0:
                    # Primer: let PE observe the xT chunk DMA once, so real
                    # matmuls never need a second (DMA) wait.
                    ci = t // 25
                    for sfx in ("a", "b"):
                        dd = inst[sfx]
                        e = (ci + 1) * 25 * dd["w"]
                        nc.tensor.matmul(scr[:], dd["xT"][:, e - 1:e],
                                         dd["xT"][:, e - 1:e],
                                         start=True, stop=True)
                rec_step("a", t)
                rec_step("b", t)
                if (t + 1) % 4 == 0:
                    c0 = (t - 3) * WA
                    mu_chunk("a", t // 4, c0, 4 * WA)
                if (t + 1) % 8 == 0:
                    c0 = (t - 7) * WB
                    mu_chunk("b", t // 8, c0, 8 * WB)
            # Tail for B: T=100 -> 12 chunks of 512 cover 96 steps; 4 left.
            rem = T % 8
            if rem:
                mu_chunk("b", 12, (T - rem) * WB, rem * WB)

    nc.compile()
    _NC_CACHE["nc"] = nc
    return nc


def _prepare_in_maps(inputs):
    folded = _fold_weights(inputs)
    x = np.asarray(inputs["x"], np.float32)

    def piece_inputs(sfx, c, b0, w):
        U, ub, M, N, V, vb = folded[c]
        xs = x[c, :, b0:b0 + w, :]                      # [T, w, F]
        xT = np.ascontiguousarray(xs.transpose(2, 0, 1)).reshape(128, T * w)
        blob = np.concatenate([U, M[:128], M[128:], N[:128], N[128:], V], axis=1)
        ubp = np.stack([ub[:128], ub[128:]], axis=1)    # [128, 2]
        return {
            f"xT_{sfx}": xT.astype(BF16),
            f"wb_{sfx}": blob.astype(np.float32).astype(BF16),
            f"ub_{sfx}": ubp.astype(np.float32),
        }

    in_maps = []
    for core in range(8):
        m = {}
        ca, ba = PIECES_A[core]
        m.update(piece_inputs("a", ca, ba, WA))
        cb, bb = PIECES_B[core]
        m.update(piece_inputs("b", cb, bb, WB))
        in_maps.append(m)
    return in_maps, folded


def kernel(**inputs):
    from concourse.bass_utils import run_bass_kernel_spmd

    in_maps, folded = _prepare_in_maps(inputs)
    nc = _build_nc()
    trace = bool(int(os.environ.get("BASS_KERNEL_TRACE", "0")))
    res = run_bass_kernel_spmd(nc, in_maps, core_ids=list(range(8)),
                               trace=trace)
    kernel.last_results = res

    out = np.empty((C, T, B, F), np.float32)
    for core in range(8):
        for sfx, w, (c, b0) in (("a", WA, PIECES_A[core]),
                                ("b", WB, PIECES_B[core])):
            muT = np.asarray(res.results[core][f"out_{sfx}"]).astype(np.float32)
            vb = folded[c][5].astype(np.float32)
            out[c, :, b0:b0 + w, :] = (muT.reshape(128, T, w).transpose(1, 2, 0)
                                       + vb)
    return out



# revision 3
# speedup vs baseline: 8467.2311x; 8467.2311x over previous
"""Trainium2 Bass kernel for MCRNNVAE eval forward — v2.

Same folded-RNN algebra and sharding as v1 (see kernel.py), plus:

- ONE merged tanh per instance per step over [128, 2w] (both h blocks),
  instead of two biased activations: the per-block bias ub is folded into
  the PE as a K=2 matmul (stationary = [ub0; ub1] rows, moving = 0/1
  indicator mask), so the ACT instruction needs no bias and can span both
  j-blocks.  Halves ACT instruction count and its fixed overhead.
- h history stored interleaved per step ([h0 | h1] blocks of one tile) so
  the merged ACT writes one contiguous range; the mu-phase N-matmuls read
  h through a strided (step-major) AP view.
- psum pool for A triple-buffered so next-step U/bias matmuls run ahead
  of the serial tanh chain.
"""

import os
import numpy as np

import ml_dtypes

C, T, B, F = 3, 100, 512, 128
H = 256
WA, WB = 128, 64  # per-core recurrence widths (columns of batch x channel)

PIECES_A = [(0, 0), (0, 128), (0, 256), (0, 384), (1, 0), (1, 128), (2, 0), (2, 128)]
PIECES_B = [(1, 256), (1, 320), (1, 384), (1, 448),
            (2, 256), (2, 320), (2, 384), (2, 448)]

BF16 = ml_dtypes.bfloat16


def _fold_weights(inputs):
    """Collapse the linear chain per channel, in float64. Returns per-channel
    (U [128,256], ub [256], M [256,256], N [256,128], V [128,128], vb [128])."""
    HX, HZ, EH, L = 128, 128, 128, 64
    g = lambda k: np.asarray(inputs[k], np.float64)
    out = []
    for c in range(C):
        Wx, bx = g("phi_x_W")[c], g("phi_x_b")[c]
        We, be = g("enc_W")[c], g("enc_b")[c]
        Wqm, bqm = g("enc_mu_W")[c], g("enc_mu_b")[c]
        Wz, bz = g("phi_z_W"), g("phi_z_b")
        Wd, bd = g("dec_W")[c], g("dec_b")[c]
        Wpm, bpm = g("dec_mu_W")[c], g("dec_mu_b")[c]
        Wih, Whh = g("rnn_Wih"), g("rnn_Whh")
        bih, bhh = g("rnn_bih"), g("rnn_bhh")

        We_x, We_h = We[:HX], We[HX:]
        Wd_z, Wd_h = Wd[:HZ], Wd[HZ:]
        Wih_x, Wih_z = Wih[:HX], Wih[HX:]

        PWz = Wqm @ Wz                     # [EH, HZ]
        P = We_x @ PWz                     # [HX, HZ]
        Q = We_h @ PWz                     # [H, HZ]
        r = be @ PWz + bqm @ Wz + bz       # [HZ]

        G = Wih_x + P @ Wih_z              # [HX, H]
        M = Q @ Wih_z + Whh                # [H, H]
        gv = r @ Wih_z + bih + bhh         # [H]

        U = Wx @ G                         # [F, H]
        ub = bx @ G + gv                   # [H]

        W2 = Wd_z @ Wpm                    # [HZ, F]
        V = Wx @ (P @ W2)                  # [F, F]
        N = (Q @ Wd_z + Wd_h) @ Wpm        # [H, F]
        vb = bx @ (P @ W2) + r @ W2 + bd @ Wpm + bpm  # [F]
        out.append((U, ub, M, N, V, vb))
    return out


_NC_CACHE = {}


def _build_nc():
    if "nc" in _NC_CACHE:
        return _NC_CACHE["nc"]
    import concourse.bacc as bacc
    import concourse.mybir as mybir
    import concourse.tile as tile

    DT = mybir.dt.bfloat16
    F32 = mybir.dt.float32
    Tanh = mybir.ActivationFunctionType.Tanh

    nc = bacc.Bacc()

    dram = {}
    for sfx, w in (("a", WA), ("b", WB)):
        dram[f"xT_{sfx}"] = nc.declare_dram_parameter(f"xT_{sfx}", [128, T * w], DT,
                                                      isOutput=False)
        dram[f"wb_{sfx}"] = nc.declare_dram_parameter(f"wb_{sfx}", [128, 1152], DT,
                                                      isOutput=False)
        # ub lhsT rows + 0/1 indicator mask, zero-padded to K=128 so the
        # bias matmul keeps FWL (fast weight load needs NumWeights==128;
        # a K=2 matmul runs its LDWEIGHTS at full rate and also breaks the
        # following matmul's FWL).  Rows 0/1 carry ubT / the indicator,
        # rows 2..127 are zero.  cols [0:128) = ubT, [128:128+2w) = ind.
        dram[f"ubi_{sfx}"] = nc.declare_dram_parameter(
            f"ubi_{sfx}", [128, 128 + 2 * w], DT, isOutput=False)
        dram[f"out_{sfx}"] = nc.declare_dram_parameter(
            f"out_{sfx}", [128, T * w], DT, isOutput=True)

    with tile.TileContext(nc) as tc:
        with (
            tc.tile_pool(name="wts", bufs=1) as wpool,
            tc.tile_pool(name="big", bufs=1) as xpool,
            tc.tile_pool(name="mu_out", bufs=1) as mupool,
            tc.tile_pool(name="ps_a", bufs=2, space="PSUM") as ps_a,
            tc.tile_pool(name="ps_b", bufs=2, space="PSUM") as ps_b,
            tc.tile_pool(name="ps_mu", bufs=3, space="PSUM") as ps_mu,
            tc.tile_pool(name="ps_scr", bufs=1, space="PSUM") as ps_scr,
        ):
            inst = {}
            scr = ps_scr.tile([1, 1], mybir.dt.float32, tag="scr", name="scr")
            gscr = wpool.tile([128, 1], mybir.dt.bfloat16, tag="gscr", name="gscr")
            for sfx, w, pspool in (("a", WA, ps_a), ("b", WB, ps_b)):
                d = {}
                blob = wpool.tile([128, 1152], DT, tag=f"wb{sfx}", name=f"wb{sfx}")
                nc.sync.dma_start(blob[:], dram[f"wb_{sfx}"][:])
                d["U"] = blob[:, 0:256]
                d["M0"] = blob[:, 256:512]
                d["M1"] = blob[:, 512:768]
                d["N0"] = blob[:, 768:896]
                d["N1"] = blob[:, 896:1024]
                d["V"] = blob[:, 1024:1152]
                ubi = wpool.tile([128, 128 + 2 * w], DT, tag=f"ubi{sfx}",
                                 name=f"ubi{sfx}")
                nc.sync.dma_start(ubi[:], dram[f"ubi_{sfx}"][:])
                d["ubT"] = ubi[:, 0:128]
                d["ind"] = ubi[:, 128:128 + 2 * w]

                # Primers: every compute instruction may carry at most ONE
                # sync wait, so let PE observe the init DMAs once via dummy
                # matmuls (weight blob + ubi).
                nc.tensor.matmul(scr[:], blob[:, 0:1], blob[:, 0:1],
                                 start=True, stop=True)
                nc.tensor.matmul(scr[:], ubi[0:1, 0:1], ubi[0:1, 0:1],
                                 start=True, stop=True)

                cols = T * w
                d["xT"] = xpool.tile([128, cols], DT, tag=f"xT{sfx}", name=f"xT{sfx}")
                nchunk = 4
                cw = cols // nchunk
                for i in range(nchunk):
                    nc.sync.dma_start(d["xT"][:, i * cw:(i + 1) * cw],
                                      dram[f"xT_{sfx}"][:, i * cw:(i + 1) * cw])
                # h history: block t (cols [t*2w,(t+1)*2w)) holds
                # [h0_t | h1_t] (transposed, bf16).  Block 0 (h_0 = 0) is
                # never touched: step 0 and mu chunk 0 special-case it away.
                d["h"] = xpool.tile([128, (T + 1) * 2 * w], DT, tag=f"h{sfx}",
                                    name=f"h{sfx}")
                d["w"] = w
                d["pspool"] = pspool
                inst[sfx] = d

            def rec_step(sfx, t):
                d = inst[sfx]
                w = d["w"]
                ps = d["pspool"].tile([128, 2 * w], mybir.dt.float32,
                                      tag=f"ps{sfx}", name=f"ps{sfx}")
                s = slice(t * w, (t + 1) * w)
                # Bias as rank-2 outer product over the whole [0:2w) range.
                nc.tensor.matmul(ps[:], d["ubT"], d["ind"],
                                 start=True, stop=False)
                hb = t * 2 * w
                for j in (0, 1):
                    js = slice(j * 128, (j + 1) * 128)
                    po = slice(j * w, (j + 1) * w)
                    nc.tensor.matmul(ps[:, po], d["U"][:, js], d["xT"][:, s],
                                     start=False, stop=(t == 0))
                    if t > 0:
                        nc.tensor.matmul(ps[:, po], d["M0"][:, js],
                                         d["h"][:, hb:hb + w],
                                         start=False, stop=False)
                        nc.tensor.matmul(ps[:, po], d["M1"][:, js],
                                         d["h"][:, hb + w:hb + 2 * w],
                                         start=False, stop=True)
                ho = (t + 1) * 2 * w
                nc.scalar.activation(d["h"][:, ho:ho + 2 * w], ps[:], Tanh)

            # mu matmuls are emitted PER STEP (3 short matmuls right after
            # the step's recurrence matmuls) so PE's strict-FIFO queue never
            # blocks the tanh chain behind a long N=512 matmul.  The psum
            # chunk tile still spans `nstep` steps; V of the chunk's first
            # step carries start=True, N1 of each step closes its range.
            MU_STEPS = {"a": 4, "b": 8}

            def mu_step(sfx, t):
                d = inst[sfx]
                w = d["w"]
                nstep = MU_STEPS[sfx]
                s0 = (t // nstep) * nstep
                if s0 == 96 and sfx == "b":
                    nstep = T - s0  # tail chunk
                k = t - s0
                ps = inst.setdefault(f"psmu_{sfx}{s0}", ps_mu.tile(
                    [128, nstep * w], mybir.dt.float32, tag="psmu",
                    name=f"psmu{sfx}"))
                po = slice(k * w, (k + 1) * w)
                cs = slice(t * w, (t + 1) * w)
                nc.tensor.matmul(ps[:, po], d["V"], d["xT"][:, cs],
                                 start=(k == 0), stop=(t == 0))
                # h_0 = 0 and block 0 of h is uninitialized: step 0's mu has
                # no N terms (mu_0 = V x_0 exactly).
                if t > 0:
                    hb = t * 2 * w
                    nc.tensor.matmul(ps[:, po], d["N0"],
                                     d["h"][:, hb:hb + w],
                                     start=False, stop=False)
                    nc.tensor.matmul(ps[:, po], d["N1"],
                                     d["h"][:, hb + w:hb + 2 * w],
                                     start=False, stop=True)
                if k == nstep - 1:
                    c0 = s0 * w
                    cw = nstep * w
                    ot = mupool.tile([128, cw], DT, tag=f"mu{sfx}{s0}",
                                     name=f"mu{sfx}{s0}")
                    nc.vector.tensor_copy(ot[:], ps[:])
                    # GPSIMD interposer: observe the DVE copy in GPSIMD
                    # program order so the out-DMA instruction needs only its
                    # queue wait.
                    nc.gpsimd.tensor_copy(gscr[:], ot[:, 0:1])
                    nc.gpsimd.dma_start(dram[f"out_{sfx}"][:, c0:c0 + cw],
                                        ot[:])

            for t in range(T):
                if t % 25 == 0:
                    # Primer: let PE observe the xT chunk DMA once, so real
                    # matmuls never need a second (DMA) wait.
                    ci = t // 25
                    for sfx in ("a", "b"):
                        dd = inst[sfx]
                        e = (ci + 1) * 25 * dd["w"]
                        nc.tensor.matmul(scr[:], dd["xT"][:, e - 1:e],
                                         dd["xT"][:, e - 1:e],
                                         start=True, stop=True)
                rec_step("a", t)
                mu_step("a", t)
                rec_step("b", t)
                mu_step("b", t)

    nc.compile()
    _NC_CACHE["nc"] = nc
    return nc


def _prepare_in_maps(inputs):
    folded = _fold_weights(inputs)
    x = np.asarray(inputs["x"], np.float32)

    def piece_inputs(sfx, c, b0, w):
        U, ub, M, N, V, vb = folded[c]
        xs = x[c, :, b0:b0 + w, :]                      # [T, w, F]
        xT = np.ascontiguousarray(xs.transpose(2, 0, 1)).reshape(128, T * w)
        blob = np.concatenate([U, M[:128], M[128:], N[:128], N[128:], V], axis=1)
        ubi = np.zeros((128, 128 + 2 * w), np.float64)
        ubi[0, :128] = ub[:128]
        ubi[1, :128] = ub[128:]
        ubi[0, 128:128 + w] = 1.0
        ubi[1, 128 + w:128 + 2 * w] = 1.0
        return {
            f"xT_{sfx}": xT.astype(BF16),
            f"wb_{sfx}": blob.astype(np.float32).astype(BF16),
            f"ubi_{sfx}": ubi.astype(np.float32).astype(BF16),
        }

    in_maps = []
    for core in range(8):
        m = {}
        ca, ba = PIECES_A[core]
        m.update(piece_inputs("a", ca, ba, WA))
        cb, bb = PIECES_B[core]
        m.update(piece_inputs("b", cb, bb, WB))
        in_maps.append(m)
    return in_maps, folded


def kernel(**inputs):
    from concourse.bass_utils import run_bass_kernel_spmd

    in_maps, folded = _prepare_in_maps(inputs)
    nc = _build_nc()
    trace = bool(int(os.environ.get("BASS_KERNEL_TRACE", "0")))
    tmpdir = os.environ.get("BASS_KERNEL_TMPDIR") or None
    res = run_bass_kernel_spmd(nc, in_maps, core_ids=list(range(8)),
                               trace=trace, tmpdir=tmpdir)
    kernel.last_results = res

    out = np.empty((C, T, B, F), np.float32)
    for core in range(8):
        for sfx, w, (c, b0) in (("a", WA, PIECES_A[core]),
                                ("b", WB, PIECES_B[core])):
            muT = np.asarray(res.results[core][f"out_{sfx}"]).astype(np.float32)
            vb = folded[c][5].astype(np.float32)
            out[c, :, b0:b0 + w, :] = (muT.reshape(128, T, w).transpose(1, 2, 0)
                                       + vb)
    return out
